# revision 14
# baseline (speedup 1.0000x reference)
"""Trainium2 Bass kernel for LlamaRALAAttention (B=2, S=4096, HID=2048, NH=16, NKV=4, HD=128).

Sharding: 8 cores = DP(batch=2) x TP(kv-head groups=4). Core c handles batch c//4,
kv group c%4 (4 q heads + 1 kv head). Softmax/mean over S stay core-local.
o_proj partials are summed on host (the only cross-core reduction).

Pipeline (per core, "everything transposed" layout):
  xT [HID,S] host-pretransposed, bf16. Projections stream xT chunks as moving operand.
  q path in [d,s] layout: q^T = Wq_h^T @ xT, RoPE via R-matmul + cos/sin mults,
    kappa=exp(min(x,0))+max(x,0) -> QkT (bf16, resident).
  k/v path in [s,d] layout: lhsT=xT tile (stationary), rhs=[Wk|Wv]; RoPE on free dim;
    kappa -> Kk_sd, v_sd (bf16, resident). KkT via PE transpose.
  Qg = mean_s Qk (DVE free-dim reduce); logits via per-s-tile matvecs (lhsT=KkT tile);
  softmax with exact global max (PE transpose + ones-matmul broadcasts, all on-chip);
  outer = (alpha*Kk)^T @ v (PE accumulate); result^T = outer^T.T... lhsT=outer, rhs=QkT;
  ctx^T = phiT * result^T; o_proj: lhsT=ctx^T tiles, rhs=Wo rows -> partial out [S, 2048].
"""

import sys

sys.path.insert(0, "/opt/trn_rl_repo")

import numpy as np
import ml_dtypes

import concourse.bass as bass
import concourse.mybir as mybir
import concourse.tile as tile
from concourse import bacc
from concourse.bass_utils import run_bass_kernel_spmd
from concourse.masks import make_identity

P = 128
S = 4096
HID = 2048
HD = 128
NHL = 4            # q heads per core
KO = HID // P      # 16 contraction subtiles
CS = 512           # token chunk size
NCH = S // CS      # 8 chunks
NST = S // P       # 32 s-tiles
ROPE_THETA = 10000.0

F32 = mybir.dt.float32
BF16 = mybir.dt.bfloat16
BF = ml_dtypes.bfloat16

_CACHE = {}


def _build():
    nc = bacc.Bacc("TRN2", target_bir_lowering=False, debug=False, num_devices=8)

    xT = nc.dram_tensor("xT", [HID, S], BF16, kind="ExternalInput").ap()
    cosT = nc.dram_tensor("cosT", [P, S], F32, kind="ExternalInput").ap()
    sinT = nc.dram_tensor("sinT", [P, S], F32, kind="ExternalInput").ap()
    cos_sd = nc.dram_tensor("cos_sd", [S, HD], F32, kind="ExternalInput").ap()
    sin_sd = nc.dram_tensor("sin_sd", [S, HD], F32, kind="ExternalInput").ap()
    Wq = nc.dram_tensor("Wq", [HID, NHL * HD], BF16, kind="ExternalInput").ap()
    Wkv = nc.dram_tensor("Wkv", [HID, 2 * HD], BF16, kind="ExternalInput").ap()
    Wphi = nc.dram_tensor("Wphi", [HID, NHL * HD], BF16, kind="ExternalInput").ap()
    Wo = nc.dram_tensor("Wo", [NHL * HD, HID], BF16, kind="ExternalInput").ap()
    bphi = nc.dram_tensor("bphi", [NHL * HD], F32, kind="ExternalInput").ap()
    RT = nc.dram_tensor("RT", [P, P], BF16, kind="ExternalInput").ap()
    out = nc.dram_tensor("out", [S, HID], F32, kind="ExternalOutput").ap()

    xT_r = xT.rearrange("(ko p) s -> p ko s", p=P)
    Wq_r = Wq.rearrange("(ko p) m -> p ko m", p=P)
    Wkv_r = Wkv.rearrange("(ko p) m -> p ko m", p=P)
    Wphi_r = Wphi.rearrange("(ko p) m -> p ko m", p=P)
    Wo_r = Wo.rearrange("(h p) n -> p h n", p=P)
    cos_sd_r = cos_sd.rearrange("(t p) d -> p t d", p=P)
    sin_sd_r = sin_sd.rearrange("(t p) d -> p t d", p=P)
    bphi_r = bphi.rearrange("(h p) -> p h", p=P)
    out_r = out.rearrange("(t p) n -> p t n", p=P)

    from contextlib import ExitStack
    with tile.TileContext(nc) as tc, ExitStack() as es:
        # ---- pools ----
        res = es.enter_context(tc.tile_pool(name="res", bufs=1))        # residents
        wts = es.enter_context(tc.tile_pool(name="wts", bufs=2))        # big weights, shared slots
        xp = es.enter_context(tc.tile_pool(name="xp", bufs=2))          # xT chunks
        stream = es.enter_context(tc.tile_pool(name="stream", bufs=2))  # big per-chunk tiles
        stream3 = es.enter_context(tc.tile_pool(name="stream3", bufs=3))  # small per-chunk tiles
        small = es.enter_context(tc.tile_pool(name="small", bufs=4))    # tiny tiles
        pq = es.enter_context(tc.tile_pool(name="pq", bufs=2, space="PSUM"))    # [128,512] proj
        pr = es.enter_context(tc.tile_pool(name="pr", bufs=2, space="PSUM"))    # [128,512] rot/result
        po = es.enter_context(tc.tile_pool(name="po", bufs=2, space="PSUM"))    # [128,512] out
        pmix = es.enter_context(tc.tile_pool(name="pmix", bufs=2, space="PSUM"))  # shared small

        # ---- residents / weights ----
        Wkv_sb = res.tile([P, KO, 2 * HD], BF16)
        nc.sync.dma_start(Wkv_sb[:], Wkv_r)
        Wq_sb = wts.tile([P, KO, NHL * HD], BF16, tag="big")
        nc.sync.dma_start(Wq_sb[:], Wq_r)
        RT_sb = res.tile([P, P], BF16)
        nc.sync.dma_start(RT_sb[:], RT)
        bphi_sb = res.tile([P, NHL], F32)
        nc.sync.dma_start(bphi_sb[:], bphi_r)

        ident_bf = res.tile([P, P], BF16)
        make_identity(nc, ident_bf[:])
        ident_f32 = res.tile([P, P], F32)
        make_identity(nc, ident_f32[:])
        ones_f32 = res.tile([P, 1], F32)
        nc.vector.memset(ones_f32[:], 1.0)
        onesr_f32 = res.tile([1, P], F32)
        nc.vector.memset(onesr_f32[:], 1.0)
        negr_f32 = res.tile([1, P], F32)
        nc.vector.memset(negr_f32[:], -1.0)

        QkT = res.tile([P, NHL, S], BF16)       # 32KB/part
        KkT = res.tile([P, S], BF16)            # 8KB/part
        Kk_sd = res.tile([P, NST, HD], BF16)    # 8KB/part
        v_sd = res.tile([P, NST, HD], BF16)     # 8KB/part
        qg_parts = res.tile([P, NHL, NCH], F32)
        outer_bf = res.tile([P, NHL, HD], BF16)
        alpha_sd = res.tile([P, NHL, NST], F32)
        logits_sd = res.tile([P, NHL, NST], F32)

        # ================= phase A: q/k/v projections + rope + kappa =================
        for c in range(NCH):
            xt = xp.tile([P, KO, CS], BF16, tag="xt")
            nc.sync.dma_start(xt[:], xT_r[:, :, c * CS:(c + 1) * CS])
            cs_t = stream.tile([P, CS], F32, tag="cosT")
            nc.sync.dma_start(cs_t[:], cosT[:, c * CS:(c + 1) * CS])
            sn_t = stream.tile([P, CS], F32, tag="sinT")
            nc.sync.dma_start(sn_t[:], sinT[:, c * CS:(c + 1) * CS])
            csd = stream.tile([P, 4, HD], F32, tag="cossd")
            nc.sync.dma_start(csd[:], cos_sd_r[:, c * 4:(c + 1) * 4, :])
            ssd = stream.tile([P, 4, HD], F32, tag="sinsd")
            nc.sync.dma_start(ssd[:], sin_sd_r[:, c * 4:(c + 1) * 4, :])

            # ---- k + v for the 4 s-tiles of this chunk ----
            for st in range(4):
                stg = c * 4 + st
                pskv = pmix.tile([P, 2 * HD], F32, tag="mix")
                for ko in range(KO):
                    nc.tensor.matmul(
                        pskv[:], xt[:, ko, st * P:(st + 1) * P], Wkv_sb[:, ko, :],
                        start=(ko == 0), stop=(ko == KO - 1))
                k_ps = pskv[:, :HD]
                nc.vector.tensor_copy(v_sd[:, stg, :], pskv[:, HD:])
                # rope-k in [s,d]: rot on free halves
                kr = stream3.tile([P, HD], F32, tag="kr")
                nc.vector.tensor_mul(kr[:], k_ps, csd[:, st, :])
                ta = stream3.tile([P, 64], F32, tag="ta")
                nc.vector.tensor_mul(ta[:], k_ps[:, 64:], ssd[:, st, :64])
                nc.vector.tensor_sub(kr[:, :64], kr[:, :64], ta[:])
                tb = stream3.tile([P, 64], F32, tag="tb")
                nc.vector.tensor_mul(tb[:], k_ps[:, :64], ssd[:, st, 64:])
                nc.vector.tensor_add(kr[:, 64:], kr[:, 64:], tb[:])
                # kappa
                mk = stream3.tile([P, HD], F32, tag="mk")
                nc.gpsimd.tensor_scalar_min(mk[:], kr[:], 0.0)
                ek = stream3.tile([P, HD], F32, tag="ek")
                nc.scalar.activation(ek[:], mk[:], mybir.ActivationFunctionType.Exp)
                nc.vector.scalar_tensor_tensor(
                    Kk_sd[:, stg, :], kr[:], 0.0, ek[:],
                    mybir.AluOpType.max, mybir.AluOpType.add)
                # KkT via PE transpose (bf16 in -> fp32 psum -> bf16 sbuf)
                pst = pmix.tile([P, P], BF16, tag="mix")
                nc.tensor.transpose(pst[:], Kk_sd[:, stg, :], ident_bf[:])
                nc.vector.tensor_copy(KkT[:, stg * P:(stg + 1) * P], pst[:])

            # ---- q heads ----
            for h in range(NHL):
                psq = pq.tile([P, CS], F32, tag="psq")
                for ko in range(KO):
                    nc.tensor.matmul(
                        psq[:], Wq_sb[:, ko, h * HD:(h + 1) * HD], xt[:, ko, :],
                        start=(ko == 0), stop=(ko == KO - 1))
                qbf = stream3.tile([P, CS], BF16, tag="qbf")
                nc.scalar.copy(qbf[:], psq[:])
                psr = pr.tile([P, CS], F32, tag="psr")
                nc.tensor.matmul(psr[:], RT_sb[:], qbf[:], start=True, stop=True)
                # qrope = q*cos + rot*sin
                t1 = stream.tile([P, CS], F32, tag="t1")
                nc.vector.tensor_mul(t1[:], psr[:], sn_t[:])
                qro = stream.tile([P, CS], F32, tag="qro")
                nc.vector.tensor_mul(qro[:], psq[:], cs_t[:])
                nc.vector.tensor_add(qro[:], qro[:], t1[:])
                # kappa -> QkT
                mq = stream.tile([P, CS], F32, tag="mq")
                nc.gpsimd.tensor_scalar_min(mq[:], qro[:], 0.0)
                eq = stream.tile([P, CS], F32, tag="eq")
                nc.scalar.activation(eq[:], mq[:], mybir.ActivationFunctionType.Exp)
                nc.vector.scalar_tensor_tensor(
                    QkT[:, h, c * CS:(c + 1) * CS], qro[:], 0.0, eq[:],
                    mybir.AluOpType.max, mybir.AluOpType.add)
                # Qg partial
                nc.vector.tensor_reduce(
                    qg_parts[:, h, c:c + 1], QkT[:, h, c * CS:(c + 1) * CS],
                    mybir.AxisListType.X, mybir.AluOpType.add)

        Wphi_sb = wts.tile([P, KO, NHL * HD], BF16, tag="big")
        nc.sync.dma_start(Wphi_sb[:], Wphi_r)
        Wo_sb = wts.tile([P, NHL, HID], BF16, tag="big")
        nc.sync.dma_start(Wo_sb[:], Wo_r)

        # ================= phase B: Qg, logits, softmax, outer =================
        qg_bf = small.tile([P, NHL], BF16, tag="qgbf")
        qg_f = small.tile([P, NHL], F32, tag="qgf")
        for h in range(NHL):
            nc.vector.tensor_reduce(
                qg_f[:, h:h + 1], qg_parts[:, h, :],
                mybir.AxisListType.X, mybir.AluOpType.add)
        nc.vector.tensor_scalar_mul(qg_bf[:], qg_f[:], 1.0 / S)

        # logits[s] per head: lhsT = KkT tile [d, s-tile], rhs = qg column
        for st in range(NST):
            psl = pmix.tile([P, NHL], F32, tag="mix")
            for h in range(NHL):
                nc.tensor.matmul(
                    psl[:, h:h + 1], KkT[:, st * P:(st + 1) * P],
                    qg_bf[:, h:h + 1], start=True, stop=True)
            nc.vector.tensor_copy(
                logits_sd.rearrange("p h t -> p t h")[:, st, :], psl[:])

        for h in range(NHL):
            lg = logits_sd[:, h, :]                       # [128, 32]
            pmax = small.tile([P, 1], F32, tag="pmax")
            nc.vector.tensor_reduce(pmax[:], lg, mybir.AxisListType.X, mybir.AluOpType.max)
            # global max: transpose pmax -> [1,128], reduce, negate-broadcast back
            pmt = pmix.tile([1, P], F32, tag="mix")
            nc.tensor.transpose(pmt[:], pmax[:], ident_f32[:])
            gmax = small.tile([1, 1], F32, tag="gmax")
            nc.vector.tensor_reduce(gmax[:], pmt[:], mybir.AxisListType.X, mybir.AluOpType.max)
            pngm = pmix.tile([P, 1], F32, tag="mix")
            nc.tensor.matmul(pngm[:], negr_f32[:], gmax[:], start=True, stop=True)
            ngm = small.tile([P, 1], F32, tag="ngm")
            nc.vector.tensor_copy(ngm[:], pngm[:])
            # e = exp(l - gmax), per-partition sums via accum_out
            e_sd = small.tile([P, NST], F32, tag="esd")
            srow = small.tile([P, 1], F32, tag="srow")
            nc.scalar.activation(e_sd[:], lg, mybir.ActivationFunctionType.Exp,
                                 bias=ngm[:], accum_out=srow[:])
            # total = sum_p srow  (fp32 matmul), then rcp broadcast
            ptot = pmix.tile([1, 1], F32, tag="mix")
            nc.tensor.matmul(ptot[:], srow[:], ones_f32[:], start=True, stop=True)
            rcp = small.tile([1, 1], F32, tag="rcp")
            nc.vector.reciprocal(rcp[:], ptot[:])
            prc = pmix.tile([P, 1], F32, tag="mix")
            nc.tensor.matmul(prc[:], onesr_f32[:], rcp[:], start=True, stop=True)
            rcpb = small.tile([P, 1], F32, tag="rcpb")
            nc.vector.tensor_copy(rcpb[:], prc[:])
            nc.vector.tensor_scalar(
                alpha_sd[:, h, :], e_sd[:], rcpb[:], float(S),
                mybir.AluOpType.mult, mybir.AluOpType.mult)

        # outer[h] = sum_st (alpha*Kk_tile)^T... lhsT=KkA [s,d], rhs=v [s,f]
        for h in range(NHL):
            pso = pmix.tile([P, HD], F32, tag="mix")
            for st in range(NST):
                kka = stream3.tile([P, HD], BF16, tag="kka")
                nc.vector.tensor_scalar_mul(
                    kka[:], Kk_sd[:, st, :], alpha_sd[:, h, st:st + 1])
                nc.tensor.matmul(pso[:], kka[:], v_sd[:, st, :],
                                 start=(st == 0), stop=(st == NST - 1))
            nc.vector.tensor_copy(outer_bf[:, h, :], pso[:])

        # ================= phase C: result_attn, ctx, o_proj =================
        for c in range(NCH):
            xt = xp.tile([P, KO, CS], BF16, tag="xt")
            nc.sync.dma_start(xt[:], xT_r[:, :, c * CS:(c + 1) * CS])
            ctx_bf = stream.tile([P, NHL, CS], BF16, tag="ctx")
            for h in range(NHL):
                psp = pq.tile([P, CS], F32, tag="psq")
                for ko in range(KO):
                    nc.tensor.matmul(
                        psp[:], Wphi_sb[:, ko, h * HD:(h + 1) * HD], xt[:, ko, :],
                        start=(ko == 0), stop=(ko == KO - 1))
                phiT = stream.tile([P, CS], F32, tag="phiT")
                nc.scalar.activation(phiT[:], psp[:], mybir.ActivationFunctionType.Identity, bias=bphi_sb[:, h:h + 1])
                psr = pr.tile([P, CS], F32, tag="psr")
                nc.tensor.matmul(psr[:], outer_bf[:, h, :],
                                 QkT[:, h, c * CS:(c + 1) * CS], start=True, stop=True)
                nc.vector.tensor_mul(ctx_bf[:, h, :], phiT[:], psr[:])
            # o_proj for the 4 s-tiles of this chunk
            for st in range(4):
                stg = c * 4 + st
                for n in range(4):
                    pso2 = po.tile([P, 512], F32, tag="psout")
                    for h in range(NHL):
                        nc.tensor.matmul(
                            pso2[:], ctx_bf[:, h, st * P:(st + 1) * P],
                            Wo_sb[:, h, n * 512:(n + 1) * 512],
                            start=(h == 0), stop=(h == NHL - 1))
                    ob = stream.tile([P, 512], F32, tag="ob")
                    if (st + n) % 2 == 0:
                        nc.vector.tensor_copy(ob[:], pso2[:])
                    else:
                        nc.scalar.copy(ob[:], pso2[:])
                    nc.sync.dma_start(out_r[:, stg, n * 512:(n + 1) * 512], ob[:])

    nc.compile()
    return nc


def _host_prep(hidden_states, position_ids, Wq, Wk, Wv, Wo, Wphi, bphi):
    B = hidden_states.shape[0]
    # rope tables (match reference fp32 math)
    inv_freq = (1.0 / (ROPE_THETA ** (np.arange(0, HD, 2, dtype=np.float32) / HD))).astype(np.float32)
    in_maps = []
    Rm = np.zeros((P, P), dtype=np.float32)
    Rm[np.arange(64), np.arange(64) + 64] = -1.0
    Rm[np.arange(64) + 64, np.arange(64)] = 1.0
    RT_np = np.ascontiguousarray(Rm.T).astype(BF)
    for b in range(B):
        freqs = position_ids[b].astype(np.float32)[:, None] * inv_freq[None, :]
        emb = np.concatenate([freqs, freqs], axis=1)          # [S, 128]
        cos_b = np.cos(emb).astype(np.float32)
        sin_b = np.sin(emb).astype(np.float32)
        xT_b = np.ascontiguousarray(hidden_states[b].T).astype(BF)
        cosT_b = np.ascontiguousarray(cos_b.T)
        sinT_b = np.ascontiguousarray(sin_b.T)
        for g in range(4):
            sl4 = slice(g * 512, (g + 1) * 512)
            sl1 = slice(g * 128, (g + 1) * 128)
            in_maps.append({
                "xT": xT_b,
                "cosT": cosT_b, "sinT": sinT_b,
                "cos_sd": cos_b, "sin_sd": sin_b,
                "Wq": np.ascontiguousarray(Wq[:, sl4]).astype(BF),
                "Wkv": np.ascontiguousarray(
                    np.concatenate([Wk[:, sl1], Wv[:, sl1]], axis=1)).astype(BF),
                "Wphi": np.ascontiguousarray(Wphi[:, sl4]).astype(BF),
                "Wo": np.ascontiguousarray(Wo[sl4, :]).astype(BF),
                "bphi": np.ascontiguousarray(bphi[sl4]).astype(np.float32),
                "RT": RT_np,
            })
    return in_maps


def kernel(hidden_states, position_ids, Wq, Wk, Wv, Wo, Wphi, bphi, _trace=False):
    if "nc" not in _CACHE:
        _CACHE["nc"] = _build()
    nc = _CACHE["nc"]
    in_maps = _host_prep(np.asarray(hidden_states), np.asarray(position_ids),
                         np.asarray(Wq), np.asarray(Wk), np.asarray(Wv),
                         np.asarray(Wo), np.asarray(Wphi), np.asarray(bphi))
    res = run_bass_kernel_spmd(nc, in_maps, list(range(8)), trace=_trace)
    _CACHE["last_res"] = res
    B = hidden_states.shape[0]
    out = np.empty((B, S, HID), dtype=np.float32)
    for b in range(B):
        acc = res.results[b * 4 + 0]["out"].astype(np.float32)
        for g in range(1, 4):
            acc = acc + res.results[b * 4 + g]["out"]
        out[b] = acc
    return out
